# revision 11
# baseline (speedup 1.0000x reference)
"""DCNv3 Trainium2 Bass kernel v2 — data-parallel (1 image/core, 8 cores).

Changes vs v1 baseline (943865 ns):
  - host-side transposes: input arrives as xT [C, T] bf16, output leaves as
    [C, T] f32 (host transposes back) -> no PE transposes, no S0 stage.
  - bf16 matmul operands everywhere (fp32 PSUM accumulation).
  - p-major (tap-major) row layout for the 144 (g,p) rows: kpi0 = taps 0..7
    (128 rows), tail = tap 8 (16 rows).  Tap-8 candidate products are packed
    into one 128-row tile (8 small DMAs) so the selection runs as 11 matmuls
    per output chunk instead of 18.
  - pair-level (1024-token) elementwise ops + paired 2-bank PSUM tiles so
    PSUM->SBUF ACT copies amortize.
  - software-pipelined emission: scand(k-1) | stage(k) | gexp+apply(k-1) |
    q-muls(k), so PE/ACT/DVE overlap across pairs.
"""
import numpy as np
from contextlib import ExitStack

import concourse.bacc as bacc
import concourse.tile as tile
import concourse.mybir as mybir
import concourse.bass_utils as bass_utils

F32 = mybir.dt.float32
BF16 = mybir.dt.bfloat16
AF = mybir.ActivationFunctionType
OP = mybir.AluOpType

N_CORES = 8
NB, H, WD, C = 8, 64, 64, 256
G, GC, P = 16, 16, 9
T = H * WD              # 4096
Hp = 66
MR = 2
ROWS = Hp + 2 * MR      # 70
FS = ROWS * Hp          # 4620
NPAIR = 4
PT = 1024               # tokens per pair
EPS = 1e-6

# tap order p: dx = p//3 - 1, dy = p%3 - 1 (matches reference kgrid)
DX = [p // 3 - 1 for p in range(P)]
DY = [p % 3 - 1 for p in range(P)]
UV = [(u, v) for u in range(-2, 3) for v in range(-2, 3)]


def _r(ap, spec, **kw):
    return ap.rearrange(spec, **kw)


def _win(padflat, r0, u, v, rows=16):
    """[128, rows, 64] shifted window of a padded flat [128, FS] buffer.
    Image row h, col w live at flat (h+1+MR)*Hp + (1+w)."""
    start = (r0 + 1 + MR + u) * Hp + (1 + v)
    sl = padflat[:, start:start + rows * Hp]
    return _r(sl, "p (r c) -> p r c", c=Hp)[:, :, 0:WD]


def _win_odd(padflat, r0, u, v, rows=16):
    """Same window on the 1-element-left-shifted buffer (even base offset)."""
    start = (r0 + 1 + MR + u) * Hp + (1 + v) - 1
    sl = padflat[:, start:start + rows * Hp]
    return _r(sl, "p (r c) -> p r c", c=Hp)[:, :, 0:WD]


def build():
    nc = bacc.Bacc("TRN2", target_bir_lowering=False, debug=False,
                   enable_asserts=True, num_devices=N_CORES)

    def din(name, shape, dt=F32):
        return nc.dram_tensor(name, list(shape), dt, kind="ExternalInput").ap()

    xT_d = din("xT", [C, T], BF16)
    inWT_d = din("inWT", [C, C], BF16)
    dwdiag_d = din("dwdiag", [128, 18, 128], BF16)
    ones_d = din("ones", [128, 128], BF16)
    headWm_d = din("headWm", [C, 384], BF16)
    headWt_d = din("headWt", [C, 80], BF16)
    headBm_d = din("headBm", [384])
    headBt_d = din("headBt", [48])
    scand0_d = din("scand0", [128, 9 * 400], BF16)
    scandP_d = din("scandP", [128, 400], BF16)
    scandQ_d = din("scandQ", [16, 400], BF16)
    blk00_d = din("blk00", [128, 128], BF16)
    blk10_d = din("blk10", [16, 128], BF16)
    gexp_d = din("gexp", [128, 8, C], BF16)
    gmap_d = din("gmap", [16, C], BF16)
    outWT_d = din("outWT", [C, C], BF16)
    dwB_d = din("dwB", [C]); lnG_d = din("lnG", [C]); lnB_d = din("lnB", [C])
    inB_d = din("inB", [C]); outB_d = din("outB", [C])

    out_d = nc.dram_tensor("out", [C, T], F32, kind="ExternalOutput").ap()

    with tile.TileContext(nc) as tc, ExitStack() as ctx:
        consts = ctx.enter_context(tc.tile_pool(name="consts", bufs=1))
        big = ctx.enter_context(tc.tile_pool(name="big", bufs=1))
        work = ctx.enter_context(tc.tile_pool(name="work", bufs=1))
        ps = ctx.enter_context(tc.tile_pool(name="ps", bufs=1, space="PSUM"))

        # ---------------- constants ----------------
        inWT = consts.tile([128, 2, C], BF16)
        nc.sync.dma_start(out=inWT, in_=_r(inWT_d, "(k p) m -> p k m", p=128))
        dwdiag = consts.tile([128, 18, 128], BF16)
        nc.sync.dma_start(out=dwdiag, in_=dwdiag_d)
        ones = consts.tile([128, 128], BF16)
        nc.sync.dma_start(out=ones, in_=ones_d)
        headWm = consts.tile([128, 2, 384], BF16)
        nc.sync.dma_start(out=headWm, in_=_r(headWm_d, "(k p) m -> p k m", p=128))
        headWt = consts.tile([128, 2, 80], BF16)
        nc.sync.dma_start(out=headWt, in_=_r(headWt_d, "(k p) m -> p k m", p=128))
        scand0 = consts.tile([128, 9 * 400], BF16)
        nc.sync.dma_start(out=scand0, in_=scand0_d)
        scandP = consts.tile([128, 400], BF16)
        nc.sync.dma_start(out=scandP, in_=scandP_d)
        scandQ = consts.tile([16, 400], BF16)
        nc.sync.dma_start(out=scandQ, in_=scandQ_d)
        blk00 = consts.tile([128, 128], BF16)
        nc.sync.dma_start(out=blk00, in_=blk00_d)
        blk10 = consts.tile([16, 128], BF16)
        nc.sync.dma_start(out=blk10, in_=blk10_d)
        gexp = consts.tile([128, 8, C], BF16)
        nc.sync.dma_start(out=gexp, in_=gexp_d)
        gmap = consts.tile([16, C], BF16)
        nc.sync.dma_start(out=gmap, in_=gmap_d)
        outWT = consts.tile([128, 2, C], BF16)
        nc.sync.dma_start(out=outWT, in_=_r(outWT_d, "(k p) m -> p k m", p=128))

        def vec2(d, tagb):
            ts = []
            for ct in range(2):
                t_ = consts.tile([128, 1], F32, tag=f"{tagb}{ct}", name=f"v_{tagb}{ct}")
                nc.sync.dma_start(out=t_, in_=_r(d[ct * 128:(ct + 1) * 128], "(p o) -> p o", o=1))
                ts.append(t_)
            return ts
        dwB = vec2(dwB_d, "dwB"); lnG = vec2(lnG_d, "lnG"); lnB = vec2(lnB_d, "lnB")
        inB = vec2(inB_d, "inB"); outB = vec2(outB_d, "outB")
        headBm = []
        for sec in range(3):
            hb = consts.tile([128, 1], F32, tag=f"hB{sec}")
            nc.sync.dma_start(out=hb, in_=_r(headBm_d[sec * 128:(sec + 1) * 128], "(p o) -> p o", o=1))
            headBm.append(hb)
        headBt = []
        for sec in range(3):
            hb = consts.tile([16, 1], F32, tag=f"hBt{sec}")
            nc.sync.dma_start(out=hb, in_=_r(headBt_d[sec * 16:(sec + 1) * 16], "(p o) -> p o", o=1))
            headBt.append(hb)
        epsT = consts.tile([128, 1], F32, name="epsT")
        nc.vector.memset(epsT, EPS)

        # ---------------- padded buffers ----------------
        xTpad = [big.tile([128, FS], BF16, tag=f"xTpad{i}", name=f"xTpad{i}") for i in range(2)]
        xppad = [big.tile([128, FS], BF16, tag=f"xppad{i}", name=f"xppad{i}") for i in range(2)]
        xppod = [big.tile([128, FS], BF16, tag=f"xppod{i}", name=f"xppod{i}") for i in range(2)]
        # margin memsets: top rows, bottom rows, L/R col pads (pairwise contiguous)
        for buf in xTpad + xppad + xppod:
            nc.vector.memset(buf[:, 0:3 * Hp + 1], 0.0)
            nc.vector.memset(buf[:, 67 * Hp - 1:FS], 0.0)
            strip = _r(buf[:, 3 * Hp + 65:3 * Hp + 65 + 63 * Hp], "p (r c) -> p r c", c=Hp)[:, :, 0:2]
            nc.vector.memset(strip, 0.0)
        for buf in xppod:
            # shifted layout: pads sit at cols {64,65} of each row
            strip2 = _r(buf[:, 3 * Hp + 64:3 * Hp + 64 + 64 * Hp], "p (r c) -> p r c", c=Hp)[:, :, 0:2]
            nc.vector.memset(strip2, 0.0)
        # input DMA (interior): per ct, [128, 64, 64] window
        for ct in range(2):
            nc.sync.dma_start(out=_win(xTpad[ct], 0, 0, 0, rows=64),
                              in_=_r(xT_d[ct * 128:(ct + 1) * 128, :], "p (r c) -> p r c", c=WD))

        # ---------------- per-pair tiles ----------------
        x1 = [big.tile([128, PT], BF16, tag=f"x1{i}", bufs=1, name=f"x1_{i}") for i in range(2)]
        ysb = [big.tile([128, PT], BF16, tag=f"ysb{i}", name=f"ysb_{i}") for i in range(2)]
        y2sb = [big.tile([128, PT], BF16, tag=f"y2sb{i}", name=f"y2sb_{i}") for i in range(2)]

        def pairT(tag, np_=128, dt=BF16, bufs=1):
            return big.tile([np_, PT], dt, tag=tag, bufs=bufs, name=f"t_{tag}")

        # ---------------- stage / apply / qmul emitters ----------------
        def stage(it):
            r0 = it * 16
            # in_proj -> xppad/xppod
            for mt in range(2):
                pp = ps.tile([128, PT], F32, tag="mp", bufs=2)
                for sl in range(2):
                    for kt in range(2):
                        nc.tensor.matmul(pp[:, sl * 512:(sl + 1) * 512],
                                         inWT[:, kt, mt * 128:(mt + 1) * 128],
                                         _win(xTpad[kt], r0 + sl * 8, 0, 0, rows=8),
                                         start=(kt == 0), stop=(kt == 1))
                ppv = _r(pp, "p (r c) -> p r c", c=WD)
                nc.scalar.activation(out=_win(xppad[mt], r0, 0, 0), in_=ppv,
                                     func=AF.Identity, bias=inB[mt], scale=1.0)
                nc.scalar.activation(out=_win_odd(xppod[mt], r0, 0, 0), in_=ppv,
                                     func=AF.Identity, bias=inB[mt], scale=1.0)
            # depthwise conv
            for ct in range(2):
                cp = ps.tile([128, PT], F32, tag="mp", bufs=2)
                for sl in range(2):
                    for tap in range(9):
                        ky, kx = tap // 3, tap % 3
                        nc.tensor.matmul(cp[:, sl * 512:(sl + 1) * 512],
                                         dwdiag[:, tap * 2 + ct, :],
                                         _win(xTpad[ct], r0 + sl * 8, ky - 1, kx - 1, rows=8),
                                         start=(tap == 0), stop=(tap == 8))
                nc.scalar.activation(out=ysb[ct], in_=cp, func=AF.Identity,
                                     bias=dwB[ct], scale=1.0)
                nc.scalar.activation(out=y2sb[ct], in_=cp, func=AF.Square,
                                     bias=dwB[ct], scale=1.0)
            # LN + GELU per 512-chunk
            for c2 in range(2):
                c0, c1 = c2 * 512, (c2 + 1) * 512
                sp = ps.tile([128, PT], F32, tag="sp", bufs=1)
                for ct in range(2):
                    nc.tensor.matmul(sp[:, 0:512], ones, ysb[ct][:, c0:c1],
                                     start=(ct == 0), stop=(ct == 1))
                for ct in range(2):
                    nc.tensor.matmul(sp[:, 512:1024], ones, y2sb[ct][:, c0:c1],
                                     start=(ct == 0), stop=(ct == 1))
                mn = work.tile([128, 512], F32, tag="mn", bufs=2)
                nc.scalar.activation(out=mn, in_=sp[:, 0:512], func=AF.Copy,
                                     scale=1.0 / C)
                msq = work.tile([128, 512], F32, tag="msq", bufs=1)
                nc.scalar.activation(out=msq, in_=mn, func=AF.Square)
                var = work.tile([128, 512], F32, tag="var", bufs=1)
                nc.vector.scalar_tensor_tensor(out=var, in0=sp[:, 512:1024],
                                               scalar=1.0 / C, in1=msq,
                                               op0=OP.mult, op1=OP.subtract)
                sd = work.tile([128, 512], F32, tag="sd", bufs=1)
                nc.scalar.activation(out=sd, in_=var, func=AF.Sqrt, bias=epsT, scale=1.0)
                rstd = work.tile([128, 512], F32, tag="rstd", bufs=2)
                nc.vector.reciprocal_approx_fast(out=rstd, in_=sd)
                for ct in range(2):
                    t1 = work.tile([128, 512], F32, tag="t1", bufs=1)
                    nc.vector.tensor_sub(out=t1, in0=ysb[ct][:, c0:c1], in1=mn)
                    t2 = work.tile([128, 512], F32, tag="t2", bufs=1)
                    nc.vector.tensor_mul(out=t2, in0=t1, in1=rstd)
                    nc.scalar.activation(out=x1[ct][:, c0:c1], in_=t2,
                                         func=AF.Gelu, bias=lnB[ct], scale=lnG[ct])
            # heads (kpi0)
            offx = pairT("offx"); offy = pairT("offy"); em = pairT("em")
            for sec, dst, fn in ((0, offx, AF.Identity), (1, offy, AF.Identity),
                                 (2, em, AF.Exp)):
                hp = ps.tile([128, PT], F32, tag="mp", bufs=2)
                for sl in range(2):
                    for kt in range(2):
                        nc.tensor.matmul(hp[:, sl * 512:(sl + 1) * 512],
                                         headWm[:, kt, sec * 128:(sec + 1) * 128],
                                         x1[kt][:, sl * 512:(sl + 1) * 512],
                                         start=(kt == 0), stop=(kt == 1))
                nc.scalar.activation(out=dst, in_=hp, func=fn, bias=headBm[sec], scale=1.0)
            # head tail: sections at 32-aligned psum rows 0:16 / 32:48 / 64:80
            pt80 = ps.tile([128, PT], F32, tag="mp", bufs=2)
            for sl in range(2):
                for kt in range(2):
                    nc.tensor.matmul(pt80[0:80, sl * 512:(sl + 1) * 512],
                                     headWt[:, kt, :],
                                     x1[kt][:, sl * 512:(sl + 1) * 512],
                                     start=(kt == 0), stop=(kt == 1))
            offtx = pairT("offtx", 16); offty = pairT("offty", 16)
            emt = pairT("emt", 16)
            nc.scalar.activation(out=offtx, in_=pt80[0:16], func=AF.Identity,
                                 bias=headBt[0], scale=1.0)
            nc.scalar.activation(out=offty, in_=pt80[32:48], func=AF.Identity,
                                 bias=headBt[1], scale=1.0)
            nc.scalar.activation(out=emt, in_=pt80[64:80], func=AF.Exp,
                                 bias=headBt[2], scale=1.0)
            # softmax denominators (group sums) + normalize masks in place
            smp = ps.tile([128, PT], F32, tag="sp", bufs=1)
            for sl in range(2):
                nc.tensor.matmul(smp[:, sl * 512:(sl + 1) * 512], blk00,
                                 em[:, sl * 512:(sl + 1) * 512], start=True, stop=False)
                nc.tensor.matmul(smp[:, sl * 512:(sl + 1) * 512], blk10,
                                 emt[:, sl * 512:(sl + 1) * 512], start=False, stop=True)
            rs = pairT("rs", dt=F32)
            nc.vector.reciprocal_approx_fast(out=rs, in_=smp)
            nc.vector.tensor_mul(out=em, in0=em, in1=rs)
            nc.vector.tensor_mul(out=emt, in0=emt, in1=rs[0:16])
            # tents kpi0: tx/ty [3] with mty folded into ty in place
            def tents(off, tagb, np_=128):
                tm = pairT(tagb + "m", np_); tp = pairT(tagb + "p", np_)
                t0 = pairT(tagb + "0", np_)
                nc.vector.tensor_scalar(out=tm, in0=off, scalar1=-1.0, scalar2=0.0,
                                        op0=OP.mult, op1=OP.max)
                nc.vector.tensor_scalar(out=tp, in0=off, scalar1=0.0, scalar2=None,
                                        op0=OP.max)
                su = work.tile([np_, PT], BF16, tag=f"su{np_}", bufs=1, name="su")
                nc.vector.tensor_add(out=su, in0=tm, in1=tp)
                nc.vector.tensor_scalar(out=t0, in0=su, scalar1=-1.0, scalar2=1.0,
                                        op0=OP.mult, op1=OP.add)
                return [tm, t0, tp]
            txs = tents(offx, "tx")
            tys = tents(offy, "ty")
            for cy in range(3):
                nc.vector.tensor_mul(out=tys[cy], in0=tys[cy], in1=em)
            txt = tents(offtx, "txt", 16)
            tyt = tents(offty, "tyt", 16)
            for cy in range(3):
                nc.vector.tensor_mul(out=tyt[cy], in0=tyt[cy], in1=emt)
            return txs, tys, txt, tyt

        def qmuls(it, fam):
            txs, tys, txt, tyt = fam
            qs = [pairT(f"q{i}", bufs=1) for i in range(9)]
            qP = pairT("qP", bufs=1)
            qQ = pairT("qQ", 16, bufs=1)
            for cy in range(3):
                for cx in range(3):
                    nc.vector.tensor_mul(out=qs[cy * 3 + cx], in0=tys[cy], in1=txs[cx])
            for cy in range(3):
                for cx in range(3):
                    cidx = cy * 3 + cx
                    qt = work.tile([16, PT], BF16, tag="qt", bufs=2, name="qt")
                    nc.vector.tensor_mul(out=qt, in0=tyt[cy], in1=txt[cx])
                    if cidx < 8:
                        nc.sync.dma_start(out=qP[cidx * 16:(cidx + 1) * 16, :], in_=qt)
                    else:
                        nc.vector.tensor_copy(out=qQ, in_=qt)
            return qs, qP, qQ

        def scand_block(it, qpack):
            qs, qP, qQ = qpack
            wsb = [big.tile([128, PT], BF16, tag=f"wsb{i}", bufs=1, name=f"wsb_{i}") for i in range(3)] \
                + [big.tile([16, PT], BF16, tag="wsb3", bufs=1, name="wsb_3")]
            for mt4 in range(4):
                m0 = mt4 * 128
                msz = 128 if mt4 < 3 else 16
                wp = ps.tile([128, PT], F32, tag="wb", bufs=1)
                for sl in range(2):
                    s0, s1 = sl * 512, (sl + 1) * 512
                    for cidx in range(9):
                        nc.tensor.matmul(wp[:msz, s0:s1],
                                         scand0[:, cidx * 400 + m0:cidx * 400 + m0 + msz],
                                         qs[cidx][:, s0:s1],
                                         start=(cidx == 0), stop=False)
                    nc.tensor.matmul(wp[:msz, s0:s1], scandP[:, m0:m0 + msz],
                                     qP[:, s0:s1], start=False, stop=False)
                    nc.tensor.matmul(wp[:msz, s0:s1], scandQ[:, m0:m0 + msz],
                                     qQ[:, s0:s1], start=False, stop=True)
                nc.scalar.copy(out=wsb[mt4][:msz], in_=wp[:msz])
            return wsb

        def apply_block(it, wsb):
            r0 = it * 16
            accs = [pairT(f"acc{i}", bufs=1) for i in range(2)]
            for iuv, (u, v) in enumerate(UV):
                r = iuv * 16
                mt4, lo = r // 128, r % 128
                for ct in range(2):
                    wbp = ps.tile([128, PT], F32, tag="wb", bufs=1)
                    for sl in range(2):
                        s0, s1 = sl * 512, (sl + 1) * 512
                        if mt4 < 3:
                            nc.tensor.matmul(wbp[:, s0:s1],
                                             gexp[:, lo // 16, ct * 128:(ct + 1) * 128],
                                             wsb[mt4][:, s0:s1], start=True, stop=True)
                        else:
                            nc.tensor.matmul(wbp[:, s0:s1],
                                             gmap[:, ct * 128:(ct + 1) * 128],
                                             wsb[3][:, s0:s1], start=True, stop=True)
                    wbs = work.tile([128, PT], BF16, tag="wbs", bufs=2)
                    nc.scalar.copy(out=wbs[:, 0:512], in_=wbp[:, 0:512])
                    nc.vector.tensor_copy(out=wbs[:, 512:1024], in_=wbp[:, 512:1024])
                    if (1 + v) % 2 == 0:
                        xsv = _win(xppad[ct], r0, u, v)
                    else:
                        xsv = _win_odd(xppod[ct], r0, u, v)
                    wbv = _r(wbs, "p (r c) -> p r c", c=WD)
                    av = _r(accs[ct], "p (r c) -> p r c", c=WD)
                    if iuv == 0:
                        nc.vector.tensor_mul(out=av, in0=wbv, in1=xsv)
                    else:
                        pr = work.tile([128, PT], BF16, tag="pr", bufs=2)
                        prv = _r(pr, "p (r c) -> p r c", c=WD)
                        nc.vector.tensor_mul(out=prv, in0=wbv, in1=xsv)
                        nc.vector.tensor_add(out=av, in0=av, in1=prv)
            # out_proj + DMA out
            for mt in range(2):
                op_ = ps.tile([128, PT], F32, tag="mp", bufs=2)
                for sl in range(2):
                    for kt in range(2):
                        nc.tensor.matmul(op_[:, sl * 512:(sl + 1) * 512],
                                         outWT[:, kt, mt * 128:(mt + 1) * 128],
                                         accs[kt][:, sl * 512:(sl + 1) * 512],
                                         start=(kt == 0), stop=(kt == 1))
                osb = work.tile([128, PT], F32, tag="osb", bufs=1, name="osb")
                nc.scalar.activation(out=osb, in_=op_, func=AF.Identity,
                                     bias=outB[mt], scale=1.0)
                nc.sync.dma_start(out=out_d[mt * 128:(mt + 1) * 128, it * PT:(it + 1) * PT],
                                  in_=osb)

        # ---------------- pipelined main loop ----------------
        fam = None
        qpack = None
        for it in range(NPAIR + 1):
            wsb = scand_block(it - 1, qpack) if it > 0 else None
            fam_new = stage(it) if it < NPAIR else None
            if it > 0:
                apply_block(it - 1, wsb)
            if it < NPAIR:
                qpack = qmuls(it, fam_new)

    return nc


# ---------------- host side ----------------
_BUILT = {}


def _get_built():
    if "nc" not in _BUILT:
        nc = build()
        nc.compile()
        _BUILT["nc"] = nc
    return _BUILT["nc"]


def prep_weights(inputs):
    import ml_dtypes
    f32 = np.float32
    tobf = lambda a: np.ascontiguousarray(a).astype(ml_dtypes.bfloat16)

    dw_w = np.asarray(inputs["dw_w"], f32)
    off_w = np.asarray(inputs["off_w"], f32)
    mask_w = np.asarray(inputs["mask_w"], f32)
    off_b = np.asarray(inputs["off_b"], f32)
    mask_b = np.asarray(inputs["mask_b"], f32)
    in_w = np.asarray(inputs["in_w"], f32)
    out_w = np.asarray(inputs["out_w"], f32)

    dwdiag = np.zeros((128, 18, 128), f32)
    cl = np.arange(128)
    for tap in range(9):
        ky, kx = tap // 3, tap % 3
        for ct in range(2):
            dwdiag[cl, tap * 2 + ct, cl] = dw_w[ct * 128:(ct + 1) * 128, 0, ky, kx]

    headWm = np.zeros((C, 384), f32)
    headBm = np.zeros((384,), f32)
    headWt = np.zeros((C, 80), f32)
    headBt = np.zeros((48,), f32)
    for p in range(P):
        for g in range(G):
            wx = off_w[g * 18 + p * 2 + 0]; bx = off_b[g * 18 + p * 2 + 0]
            wy = off_w[g * 18 + p * 2 + 1]; by = off_b[g * 18 + p * 2 + 1]
            wm = mask_w[g * 9 + p];         bm = mask_b[g * 9 + p]
            if p < 8:
                r = p * 16 + g
                headWm[:, 0 * 128 + r] = wx; headBm[0 * 128 + r] = bx
                headWm[:, 1 * 128 + r] = wy; headBm[1 * 128 + r] = by
                headWm[:, 2 * 128 + r] = wm; headBm[2 * 128 + r] = bm
            else:
                headWt[:, 0 + g] = wx;  headBt[0 + g] = bx
                headWt[:, 32 + g] = wy; headBt[16 + g] = by
                headWt[:, 64 + g] = wm; headBt[32 + g] = bm

    scand0 = np.zeros((128, 9 * 400), f32)
    for p in range(8):
        for g in range(G):
            r = p * 16 + g
            for cy in range(3):
                for cx in range(3):
                    cidx = cy * 3 + cx
                    uv = (DY[p] + cy + 1) * 5 + (DX[p] + cx + 1)
                    scand0[r, cidx * 400 + uv * 16 + g] = 1.0
    scandP = np.zeros((128, 400), f32)
    scandQ = np.zeros((16, 400), f32)
    for cy in range(3):
        for cx in range(3):
            cidx = cy * 3 + cx
            uv = (DY[8] + cy + 1) * 5 + (DX[8] + cx + 1)
            for g in range(G):
                if cidx < 8:
                    scandP[cidx * 16 + g, uv * 16 + g] = 1.0
                else:
                    scandQ[g, uv * 16 + g] = 1.0

    blk00 = np.zeros((128, 128), f32)
    blk10 = np.zeros((16, 128), f32)
    for p1 in range(8):
        for p2 in range(8):
            for g in range(G):
                blk00[p1 * 16 + g, p2 * 16 + g] = 1.0
    for p2 in range(8):
        for g in range(G):
            blk10[g, p2 * 16 + g] = 1.0

    gexpm = np.zeros((128, 8, C), f32)
    for uvpos in range(8):
        for c in range(C):
            gexpm[uvpos * 16 + c // GC, uvpos, c] = 1.0
    gmapm = np.zeros((16, C), f32)
    for c in range(C):
        gmapm[c // GC, c] = 1.0

    return {
        "inWT": tobf(in_w.T),
        "dwdiag": tobf(dwdiag),
        "ones": tobf(np.ones((128, 128), f32)),
        "headWm": tobf(headWm), "headWt": tobf(headWt),
        "headBm": headBm, "headBt": headBt,
        "scand0": tobf(scand0), "scandP": tobf(scandP), "scandQ": tobf(scandQ),
        "blk00": tobf(blk00), "blk10": tobf(blk10),
        "gexp": tobf(gexpm), "gmap": tobf(gmapm),
        "outWT": tobf(out_w.T),
        "dwB": np.asarray(inputs["dw_b"], f32),
        "lnG": np.asarray(inputs["ln_g"], f32),
        "lnB": np.asarray(inputs["ln_b"], f32),
        "inB": np.asarray(inputs["in_b"], f32),
        "outB": np.asarray(inputs["out_b"], f32),
    }


def make_in_maps(inputs):
    import ml_dtypes
    wts = prep_weights(inputs)
    x = np.asarray(inputs["x"], np.float32)
    in_maps = []
    for n in range(N_CORES):
        m = dict(wts)
        m["xT"] = np.ascontiguousarray(x[n].reshape(T, C).T).astype(ml_dtypes.bfloat16)
        in_maps.append(m)
    return in_maps


def kernel(**inputs):
    nc = _get_built()
    in_maps = make_in_maps(inputs)
    res = bass_utils.run_bass_kernel_spmd(nc, in_maps, core_ids=list(range(N_CORES)))
    out = np.stack([np.asarray(res.results[n]["out"], np.float32).T.reshape(H, WD, C)
                    for n in range(N_CORES)])
    return out


# revision 13
# speedup vs baseline: 1.4477x; 1.4477x over previous
"""DCNv3 Trainium2 Bass kernel v2 — data-parallel (1 image/core, 8 cores).

Changes vs v1 baseline (943865 ns):
  - host-side transposes: input arrives as xT [C, T] bf16, output leaves as
    [C, T] f32 (host transposes back) -> no PE transposes, no S0 stage.
  - bf16 matmul operands everywhere (fp32 PSUM accumulation).
  - p-major (tap-major) row layout for the 144 (g,p) rows: kpi0 = taps 0..7
    (128 rows), tail = tap 8 (16 rows).  Tap-8 candidate products are packed
    into one 128-row tile (8 small DMAs) so the selection runs as 11 matmuls
    per output chunk instead of 18.
  - pair-level (1024-token) elementwise ops + paired 2-bank PSUM tiles so
    PSUM->SBUF ACT copies amortize.
  - software-pipelined emission: scand(k-1) | stage(k) | gexp+apply(k-1) |
    q-muls(k), so PE/ACT/DVE overlap across pairs.
"""
import numpy as np
from contextlib import ExitStack

import concourse.bacc as bacc
import concourse.tile as tile
import concourse.mybir as mybir
import concourse.bass_utils as bass_utils

F32 = mybir.dt.float32
BF16 = mybir.dt.bfloat16
AF = mybir.ActivationFunctionType
OP = mybir.AluOpType

N_CORES = 8
NB, H, WD, C = 8, 64, 64, 256
G, GC, P = 16, 16, 9
T = H * WD              # 4096
Hp = 66
MR = 2
ROWS = Hp + 2 * MR      # 70
FS = ROWS * Hp          # 4620
NPAIR = 4
PT = 1024               # tokens per pair
EPS = 1e-6

# tap order p: dx = p//3 - 1, dy = p%3 - 1 (matches reference kgrid)
DX = [p // 3 - 1 for p in range(P)]
DY = [p % 3 - 1 for p in range(P)]
UV = [(u, v) for u in range(-2, 3) for v in range(-2, 3)]


def _r(ap, spec, **kw):
    return ap.rearrange(spec, **kw)


def _win(padflat, r0, u, v, rows=16):
    """[128, rows, 64] shifted window of a padded flat [128, FS] buffer.
    Image row h, col w live at flat (h+1+MR)*Hp + (1+w)."""
    start = (r0 + 1 + MR + u) * Hp + (1 + v)
    sl = padflat[:, start:start + rows * Hp]
    return _r(sl, "p (r c) -> p r c", c=Hp)[:, :, 0:WD]


def _win_odd(padflat, r0, u, v, rows=16):
    """Same window on the 1-element-left-shifted buffer (even base offset)."""
    start = (r0 + 1 + MR + u) * Hp + (1 + v) - 1
    sl = padflat[:, start:start + rows * Hp]
    return _r(sl, "p (r c) -> p r c", c=Hp)[:, :, 0:WD]


def build():
    nc = bacc.Bacc("TRN2", target_bir_lowering=False, debug=False,
                   enable_asserts=True, num_devices=N_CORES)

    def din(name, shape, dt=F32):
        return nc.dram_tensor(name, list(shape), dt, kind="ExternalInput").ap()

    xT_d = din("xT", [C, T], BF16)
    inWT_d = din("inWT", [C, C], BF16)
    dwdiag_d = din("dwdiag", [128, 18, 128], BF16)
    ones_d = din("ones", [128, 128], BF16)
    headWm_d = din("headWm", [C, 384], BF16)
    headWt_d = din("headWt", [C, 80], BF16)
    headBm_d = din("headBm", [384])
    headBt_d = din("headBt", [48])
    scand0_d = din("scand0", [128, 9 * 400], BF16)
    scandP_d = din("scandP", [128, 400], BF16)
    scandQ_d = din("scandQ", [16, 400], BF16)
    blk00_d = din("blk00", [128, 128], BF16)
    blk10_d = din("blk10", [16, 128], BF16)
    gexp_d = din("gexp", [128, 8, C], BF16)
    gmap_d = din("gmap", [16, C], BF16)
    outWT_d = din("outWT", [C, C], BF16)
    dwB_d = din("dwB", [C]); lnG_d = din("lnG", [C]); lnB_d = din("lnB", [C])
    inB_d = din("inB", [C]); outB_d = din("outB", [C])

    out_d = nc.dram_tensor("out", [C, T], F32, kind="ExternalOutput").ap()

    with tile.TileContext(nc) as tc, ExitStack() as ctx:
        consts = ctx.enter_context(tc.tile_pool(name="consts", bufs=1))
        big = ctx.enter_context(tc.tile_pool(name="big", bufs=1))
        work = ctx.enter_context(tc.tile_pool(name="work", bufs=1))
        ps = ctx.enter_context(tc.tile_pool(name="ps", bufs=1, space="PSUM"))

        # ---------------- constants ----------------
        inWT = consts.tile([128, 2, C], BF16)
        nc.sync.dma_start(out=inWT, in_=_r(inWT_d, "(k p) m -> p k m", p=128))
        dwdiag = consts.tile([128, 18, 128], BF16)
        nc.sync.dma_start(out=dwdiag, in_=dwdiag_d)
        ones = consts.tile([128, 128], BF16)
        nc.sync.dma_start(out=ones, in_=ones_d)
        headWm = consts.tile([128, 2, 384], BF16)
        nc.sync.dma_start(out=headWm, in_=_r(headWm_d, "(k p) m -> p k m", p=128))
        headWt = consts.tile([128, 2, 80], BF16)
        nc.sync.dma_start(out=headWt, in_=_r(headWt_d, "(k p) m -> p k m", p=128))
        scand0 = consts.tile([128, 9 * 400], BF16)
        nc.sync.dma_start(out=scand0, in_=scand0_d)
        scandP = consts.tile([128, 400], BF16)
        nc.sync.dma_start(out=scandP, in_=scandP_d)
        scandQ = consts.tile([16, 400], BF16)
        nc.sync.dma_start(out=scandQ, in_=scandQ_d)
        blk00 = consts.tile([128, 128], BF16)
        nc.sync.dma_start(out=blk00, in_=blk00_d)
        blk10 = consts.tile([16, 128], BF16)
        nc.sync.dma_start(out=blk10, in_=blk10_d)
        gexp = consts.tile([128, 8, C], BF16)
        nc.sync.dma_start(out=gexp, in_=gexp_d)
        gmap = consts.tile([16, C], BF16)
        nc.sync.dma_start(out=gmap, in_=gmap_d)
        outWT = consts.tile([128, 2, C], BF16)
        nc.sync.dma_start(out=outWT, in_=_r(outWT_d, "(k p) m -> p k m", p=128))

        def vec2(d, tagb):
            ts = []
            for ct in range(2):
                t_ = consts.tile([128, 1], F32, tag=f"{tagb}{ct}", name=f"v_{tagb}{ct}")
                nc.sync.dma_start(out=t_, in_=_r(d[ct * 128:(ct + 1) * 128], "(p o) -> p o", o=1))
                ts.append(t_)
            return ts
        dwB = vec2(dwB_d, "dwB"); lnG = vec2(lnG_d, "lnG"); lnB = vec2(lnB_d, "lnB")
        inB = vec2(inB_d, "inB"); outB = vec2(outB_d, "outB")
        headBm = []
        for sec in range(3):
            hb = consts.tile([128, 1], F32, tag=f"hB{sec}")
            nc.sync.dma_start(out=hb, in_=_r(headBm_d[sec * 128:(sec + 1) * 128], "(p o) -> p o", o=1))
            headBm.append(hb)
        headBt = []
        for sec in range(3):
            hb = consts.tile([16, 1], F32, tag=f"hBt{sec}")
            nc.sync.dma_start(out=hb, in_=_r(headBt_d[sec * 16:(sec + 1) * 16], "(p o) -> p o", o=1))
            headBt.append(hb)
        epsT = consts.tile([128, 1], F32, name="epsT")
        nc.vector.memset(epsT, EPS)

        # ---------------- padded buffers ----------------
        xTpad = [big.tile([128, FS], BF16, tag=f"xTpad{i}", name=f"xTpad{i}") for i in range(2)]
        xppad = [big.tile([128, FS], BF16, tag=f"xppad{i}", name=f"xppad{i}") for i in range(2)]
        xppod = [big.tile([128, FS], BF16, tag=f"xppod{i}", name=f"xppod{i}") for i in range(2)]
        # margin memsets: top rows, bottom rows, L/R col pads (pairwise contiguous)
        for buf in xTpad + xppad + xppod:
            nc.vector.memset(buf[:, 0:3 * Hp + 1], 0.0)
            nc.vector.memset(buf[:, 67 * Hp - 1:FS], 0.0)
            strip = _r(buf[:, 3 * Hp + 65:3 * Hp + 65 + 63 * Hp], "p (r c) -> p r c", c=Hp)[:, :, 0:2]
            nc.vector.memset(strip, 0.0)
        for buf in xppod:
            # shifted layout: pads sit at cols {64,65} of each row
            strip2 = _r(buf[:, 3 * Hp + 64:3 * Hp + 64 + 64 * Hp], "p (r c) -> p r c", c=Hp)[:, :, 0:2]
            nc.vector.memset(strip2, 0.0)
        # input DMA (interior): per ct, [128, 64, 64] window
        for ct in range(2):
            nc.sync.dma_start(out=_win(xTpad[ct], 0, 0, 0, rows=64),
                              in_=_r(xT_d[ct * 128:(ct + 1) * 128, :], "p (r c) -> p r c", c=WD))

        # ---------------- per-pair tiles ----------------
        x1 = [big.tile([128, PT], BF16, tag=f"x1{i}", bufs=1, name=f"x1_{i}") for i in range(2)]
        ysb = [big.tile([128, PT], BF16, tag=f"ysb{i}", name=f"ysb_{i}") for i in range(2)]
        y2sb = [big.tile([128, PT], BF16, tag=f"y2sb{i}", name=f"y2sb_{i}") for i in range(2)]

        def pairT(tag, np_=128, dt=BF16, bufs=1):
            return big.tile([np_, PT], dt, tag=tag, bufs=bufs, name=f"t_{tag}")

        # ---------------- stage / apply / qmul emitters ----------------
        def stage(it):
            r0 = it * 16
            # in_proj -> xppad/xppod
            for mt in range(2):
                pp = ps.tile([128, PT], F32, tag="mp", bufs=2)
                for sl in range(2):
                    for kt in range(2):
                        nc.tensor.matmul(pp[:, sl * 512:(sl + 1) * 512],
                                         inWT[:, kt, mt * 128:(mt + 1) * 128],
                                         _win(xTpad[kt], r0 + sl * 8, 0, 0, rows=8),
                                         start=(kt == 0), stop=(kt == 1))
                ppv = _r(pp, "p (r c) -> p r c", c=WD)
                nc.scalar.activation(out=_win(xppad[mt], r0, 0, 0), in_=ppv,
                                     func=AF.Identity, bias=inB[mt], scale=1.0)
                nc.scalar.activation(out=_win_odd(xppod[mt], r0, 0, 0), in_=ppv,
                                     func=AF.Identity, bias=inB[mt], scale=1.0)
            # depthwise conv
            for ct in range(2):
                cp = ps.tile([128, PT], F32, tag="mp", bufs=2)
                for sl in range(2):
                    for tap in range(9):
                        ky, kx = tap // 3, tap % 3
                        nc.tensor.matmul(cp[:, sl * 512:(sl + 1) * 512],
                                         dwdiag[:, tap * 2 + ct, :],
                                         _win(xTpad[ct], r0 + sl * 8, ky - 1, kx - 1, rows=8),
                                         start=(tap == 0), stop=(tap == 8))
                nc.scalar.activation(out=ysb[ct], in_=cp, func=AF.Identity,
                                     bias=dwB[ct], scale=1.0)
                nc.scalar.activation(out=y2sb[ct], in_=cp, func=AF.Square,
                                     bias=dwB[ct], scale=1.0)
            # LN + GELU per 512-chunk
            for c2 in range(2):
                c0, c1 = c2 * 512, (c2 + 1) * 512
                sp = ps.tile([128, PT], F32, tag="mp", bufs=2)
                for ct in range(2):
                    nc.tensor.matmul(sp[:, 0:512], ones, ysb[ct][:, c0:c1],
                                     start=(ct == 0), stop=(ct == 1))
                for ct in range(2):
                    nc.tensor.matmul(sp[:, 512:1024], ones, y2sb[ct][:, c0:c1],
                                     start=(ct == 0), stop=(ct == 1))
                mn = work.tile([128, 512], F32, tag="mn", bufs=2)
                nc.scalar.activation(out=mn, in_=sp[:, 0:512], func=AF.Copy,
                                     scale=1.0 / C)
                msq = work.tile([128, 512], F32, tag="msq", bufs=1)
                nc.scalar.activation(out=msq, in_=mn, func=AF.Square)
                var = work.tile([128, 512], F32, tag="var", bufs=1)
                nc.vector.scalar_tensor_tensor(out=var, in0=sp[:, 512:1024],
                                               scalar=1.0 / C, in1=msq,
                                               op0=OP.mult, op1=OP.subtract)
                sd = work.tile([128, 512], F32, tag="sd", bufs=1)
                nc.scalar.activation(out=sd, in_=var, func=AF.Sqrt, bias=epsT, scale=1.0)
                rstd = work.tile([128, 512], F32, tag="rstd", bufs=2)
                nc.vector.reciprocal_approx_fast(out=rstd, in_=sd)
                for ct in range(2):
                    t1 = work.tile([128, 512], F32, tag="t1", bufs=1)
                    nc.vector.tensor_sub(out=t1, in0=ysb[ct][:, c0:c1], in1=mn)
                    t2 = work.tile([128, 512], F32, tag="t2", bufs=1)
                    nc.vector.tensor_mul(out=t2, in0=t1, in1=rstd)
                    nc.scalar.activation(out=x1[ct][:, c0:c1], in_=t2,
                                         func=AF.Gelu, bias=lnB[ct], scale=lnG[ct])
            # heads (kpi0)
            offx = pairT("offx"); offy = pairT("offy"); em = pairT("em")
            for sec, dst, fn in ((0, offx, AF.Identity), (1, offy, AF.Identity),
                                 (2, em, AF.Exp)):
                hp = ps.tile([128, PT], F32, tag="mp", bufs=2)
                for sl in range(2):
                    for kt in range(2):
                        nc.tensor.matmul(hp[:, sl * 512:(sl + 1) * 512],
                                         headWm[:, kt, sec * 128:(sec + 1) * 128],
                                         x1[kt][:, sl * 512:(sl + 1) * 512],
                                         start=(kt == 0), stop=(kt == 1))
                nc.scalar.activation(out=dst, in_=hp, func=fn, bias=headBm[sec], scale=1.0)
            # head tail: sections at 32-aligned psum rows 0:16 / 32:48 / 64:80
            pt80 = ps.tile([128, PT], F32, tag="mp", bufs=2)
            for sl in range(2):
                for kt in range(2):
                    nc.tensor.matmul(pt80[0:80, sl * 512:(sl + 1) * 512],
                                     headWt[:, kt, :],
                                     x1[kt][:, sl * 512:(sl + 1) * 512],
                                     start=(kt == 0), stop=(kt == 1))
            offtx = pairT("offtx", 16); offty = pairT("offty", 16)
            emt = pairT("emt", 16)
            nc.scalar.activation(out=offtx, in_=pt80[0:16], func=AF.Identity,
                                 bias=headBt[0], scale=1.0)
            nc.scalar.activation(out=offty, in_=pt80[32:48], func=AF.Identity,
                                 bias=headBt[1], scale=1.0)
            nc.scalar.activation(out=emt, in_=pt80[64:80], func=AF.Exp,
                                 bias=headBt[2], scale=1.0)
            # softmax denominators (group sums) + normalize masks in place
            smp = ps.tile([128, PT], F32, tag="mp", bufs=2)
            for sl in range(2):
                nc.tensor.matmul(smp[:, sl * 512:(sl + 1) * 512], blk00,
                                 em[:, sl * 512:(sl + 1) * 512], start=True, stop=False)
                nc.tensor.matmul(smp[:, sl * 512:(sl + 1) * 512], blk10,
                                 emt[:, sl * 512:(sl + 1) * 512], start=False, stop=True)
            rs = pairT("rs", dt=F32)
            nc.vector.reciprocal_approx_fast(out=rs, in_=smp)
            nc.vector.tensor_mul(out=em, in0=em, in1=rs)
            nc.vector.tensor_mul(out=emt, in0=emt, in1=rs[0:16])
            # tents kpi0: tx/ty [3] with mty folded into ty in place
            def tents(off, tagb, np_=128):
                tm = pairT(tagb + "m", np_); tp = pairT(tagb + "p", np_)
                t0 = pairT(tagb + "0", np_)
                nc.vector.tensor_scalar(out=tm, in0=off, scalar1=-1.0, scalar2=0.0,
                                        op0=OP.mult, op1=OP.max)
                nc.vector.tensor_scalar(out=tp, in0=off, scalar1=0.0, scalar2=None,
                                        op0=OP.max)
                su = work.tile([np_, PT], BF16, tag=f"su{np_}", bufs=1, name="su")
                nc.vector.tensor_add(out=su, in0=tm, in1=tp)
                nc.vector.tensor_scalar(out=t0, in0=su, scalar1=-1.0, scalar2=1.0,
                                        op0=OP.mult, op1=OP.add)
                return [tm, t0, tp]
            txs = tents(offx, "tx")
            tys = tents(offy, "ty")
            for cy in range(3):
                nc.vector.tensor_mul(out=tys[cy], in0=tys[cy], in1=em)
            txt = tents(offtx, "txt", 16)
            tyt = tents(offty, "tyt", 16)
            for cy in range(3):
                nc.vector.tensor_mul(out=tyt[cy], in0=tyt[cy], in1=emt)
            return txs, tys, txt, tyt

        def qmuls(it, fam):
            txs, tys, txt, tyt = fam
            qs = [pairT(f"q{i}", bufs=1) for i in range(9)]
            qP = pairT("qP", bufs=1)
            qQ = pairT("qQ", 16, bufs=1)
            for cy in range(3):
                for cx in range(3):
                    nc.vector.tensor_mul(out=qs[cy * 3 + cx], in0=tys[cy], in1=txs[cx])
            for cy in range(3):
                for cx in range(3):
                    cidx = cy * 3 + cx
                    qt = work.tile([16, PT], BF16, tag="qt", bufs=2, name="qt")
                    nc.vector.tensor_mul(out=qt, in0=tyt[cy], in1=txt[cx])
                    if cidx < 8:
                        nc.sync.dma_start(out=qP[cidx * 16:(cidx + 1) * 16, :], in_=qt)
                    else:
                        nc.vector.tensor_copy(out=qQ, in_=qt)
            return qs, qP, qQ

        def scand_block(it, qpack):
            qs, qP, qQ = qpack
            wsb = [big.tile([128, PT], BF16, tag=f"wsb{i}", bufs=1, name=f"wsb_{i}") for i in range(3)] \
                + [big.tile([16, PT], BF16, tag="wsb3", bufs=1, name="wsb_3")]
            for mt4 in range(4):
                m0 = mt4 * 128
                msz = 128 if mt4 < 3 else 16
                wp = ps.tile([128, PT], F32, tag="wb", bufs=2)
                for sl in range(2):
                    s0, s1 = sl * 512, (sl + 1) * 512
                    for cidx in range(9):
                        nc.tensor.matmul(wp[:msz, s0:s1],
                                         scand0[:, cidx * 400 + m0:cidx * 400 + m0 + msz],
                                         qs[cidx][:, s0:s1],
                                         start=(cidx == 0), stop=False)
                    nc.tensor.matmul(wp[:msz, s0:s1], scandP[:, m0:m0 + msz],
                                     qP[:, s0:s1], start=False, stop=False)
                    nc.tensor.matmul(wp[:msz, s0:s1], scandQ[:, m0:m0 + msz],
                                     qQ[:, s0:s1], start=False, stop=True)
                nc.scalar.copy(out=wsb[mt4][:msz], in_=wp[:msz])
            return wsb

        def apply_block(it, wsb):
            r0 = it * 16
            accs = [pairT(f"acc{i}", bufs=1) for i in range(2)]
            for iuv, (u, v) in enumerate(UV):
                r = iuv * 16
                mt4, lo = r // 128, r % 128
                for ct in range(2):
                    wbp = ps.tile([128, PT], F32, tag="wb", bufs=2)
                    for sl in range(2):
                        s0, s1 = sl * 512, (sl + 1) * 512
                        if mt4 < 3:
                            nc.tensor.matmul(wbp[:, s0:s1],
                                             gexp[:, lo // 16, ct * 128:(ct + 1) * 128],
                                             wsb[mt4][:, s0:s1], start=True, stop=True)
                        else:
                            nc.tensor.matmul(wbp[:, s0:s1],
                                             gmap[:, ct * 128:(ct + 1) * 128],
                                             wsb[3][:, s0:s1], start=True, stop=True)
                    wbs = work.tile([128, PT], BF16, tag="wbs", bufs=2)
                    nc.scalar.copy(out=wbs, in_=wbp)
                    if (1 + v) % 2 == 0:
                        xsv = _win(xppad[ct], r0, u, v)
                    else:
                        xsv = _win_odd(xppod[ct], r0, u, v)
                    wbv = _r(wbs, "p (r c) -> p r c", c=WD)
                    av = _r(accs[ct], "p (r c) -> p r c", c=WD)
                    if iuv == 0:
                        nc.vector.tensor_mul(out=av, in0=wbv, in1=xsv)
                    else:
                        pr = work.tile([128, PT], BF16, tag="pr", bufs=2)
                        prv = _r(pr, "p (r c) -> p r c", c=WD)
                        nc.vector.tensor_mul(out=prv, in0=wbv, in1=xsv)
                        nc.vector.tensor_add(out=av, in0=av, in1=prv)
            # out_proj + DMA out
            for mt in range(2):
                op_ = ps.tile([128, PT], F32, tag="mp", bufs=2)
                for sl in range(2):
                    for kt in range(2):
                        nc.tensor.matmul(op_[:, sl * 512:(sl + 1) * 512],
                                         outWT[:, kt, mt * 128:(mt + 1) * 128],
                                         accs[kt][:, sl * 512:(sl + 1) * 512],
                                         start=(kt == 0), stop=(kt == 1))
                osb = work.tile([128, PT], F32, tag="osb", bufs=1, name="osb")
                nc.scalar.activation(out=osb, in_=op_, func=AF.Identity,
                                     bias=outB[mt], scale=1.0)
                nc.sync.dma_start(out=out_d[mt * 128:(mt + 1) * 128, it * PT:(it + 1) * PT],
                                  in_=osb)

        # ---------------- pipelined main loop ----------------
        fam = None
        qpack = None
        for it in range(NPAIR + 1):
            wsb = scand_block(it - 1, qpack) if it > 0 else None
            fam_new = stage(it) if it < NPAIR else None
            if it > 0:
                apply_block(it - 1, wsb)
            if it < NPAIR:
                qpack = qmuls(it, fam_new)

    return nc


# ---------------- host side ----------------
_BUILT = {}


def _get_built():
    if "nc" not in _BUILT:
        nc = build()
        nc.compile()
        _BUILT["nc"] = nc
    return _BUILT["nc"]


def prep_weights(inputs):
    import ml_dtypes
    f32 = np.float32
    tobf = lambda a: np.ascontiguousarray(a).astype(ml_dtypes.bfloat16)

    dw_w = np.asarray(inputs["dw_w"], f32)
    off_w = np.asarray(inputs["off_w"], f32)
    mask_w = np.asarray(inputs["mask_w"], f32)
    off_b = np.asarray(inputs["off_b"], f32)
    mask_b = np.asarray(inputs["mask_b"], f32)
    in_w = np.asarray(inputs["in_w"], f32)
    out_w = np.asarray(inputs["out_w"], f32)

    dwdiag = np.zeros((128, 18, 128), f32)
    cl = np.arange(128)
    for tap in range(9):
        ky, kx = tap // 3, tap % 3
        for ct in range(2):
            dwdiag[cl, tap * 2 + ct, cl] = dw_w[ct * 128:(ct + 1) * 128, 0, ky, kx]

    headWm = np.zeros((C, 384), f32)
    headBm = np.zeros((384,), f32)
    headWt = np.zeros((C, 80), f32)
    headBt = np.zeros((48,), f32)
    for p in range(P):
        for g in range(G):
            wx = off_w[g * 18 + p * 2 + 0]; bx = off_b[g * 18 + p * 2 + 0]
            wy = off_w[g * 18 + p * 2 + 1]; by = off_b[g * 18 + p * 2 + 1]
            wm = mask_w[g * 9 + p];         bm = mask_b[g * 9 + p]
            if p < 8:
                r = p * 16 + g
                headWm[:, 0 * 128 + r] = wx; headBm[0 * 128 + r] = bx
                headWm[:, 1 * 128 + r] = wy; headBm[1 * 128 + r] = by
                headWm[:, 2 * 128 + r] = wm; headBm[2 * 128 + r] = bm
            else:
                headWt[:, 0 + g] = wx;  headBt[0 + g] = bx
                headWt[:, 32 + g] = wy; headBt[16 + g] = by
                headWt[:, 64 + g] = wm; headBt[32 + g] = bm

    scand0 = np.zeros((128, 9 * 400), f32)
    for p in range(8):
        for g in range(G):
            r = p * 16 + g
            for cy in range(3):
                for cx in range(3):
                    cidx = cy * 3 + cx
                    uv = (DY[p] + cy + 1) * 5 + (DX[p] + cx + 1)
                    scand0[r, cidx * 400 + uv * 16 + g] = 1.0
    scandP = np.zeros((128, 400), f32)
    scandQ = np.zeros((16, 400), f32)
    for cy in range(3):
        for cx in range(3):
            cidx = cy * 3 + cx
            uv = (DY[8] + cy + 1) * 5 + (DX[8] + cx + 1)
            for g in range(G):
                if cidx < 8:
                    scandP[cidx * 16 + g, uv * 16 + g] = 1.0
                else:
                    scandQ[g, uv * 16 + g] = 1.0

    blk00 = np.zeros((128, 128), f32)
    blk10 = np.zeros((16, 128), f32)
    for p1 in range(8):
        for p2 in range(8):
            for g in range(G):
                blk00[p1 * 16 + g, p2 * 16 + g] = 1.0
    for p2 in range(8):
        for g in range(G):
            blk10[g, p2 * 16 + g] = 1.0

    gexpm = np.zeros((128, 8, C), f32)
    for uvpos in range(8):
        for c in range(C):
            gexpm[uvpos * 16 + c // GC, uvpos, c] = 1.0
    gmapm = np.zeros((16, C), f32)
    for c in range(C):
        gmapm[c // GC, c] = 1.0

    return {
        "inWT": tobf(in_w.T),
        "dwdiag": tobf(dwdiag),
        "ones": tobf(np.ones((128, 128), f32)),
        "headWm": tobf(headWm), "headWt": tobf(headWt),
        "headBm": headBm, "headBt": headBt,
        "scand0": tobf(scand0), "scandP": tobf(scandP), "scandQ": tobf(scandQ),
        "blk00": tobf(blk00), "blk10": tobf(blk10),
        "gexp": tobf(gexpm), "gmap": tobf(gmapm),
        "outWT": tobf(out_w.T),
        "dwB": np.asarray(inputs["dw_b"], f32),
        "lnG": np.asarray(inputs["ln_g"], f32),
        "lnB": np.asarray(inputs["ln_b"], f32),
        "inB": np.asarray(inputs["in_b"], f32),
        "outB": np.asarray(inputs["out_b"], f32),
    }


def make_in_maps(inputs):
    import ml_dtypes
    wts = prep_weights(inputs)
    x = np.asarray(inputs["x"], np.float32)
    in_maps = []
    for n in range(N_CORES):
        m = dict(wts)
        m["xT"] = np.ascontiguousarray(x[n].reshape(T, C).T).astype(ml_dtypes.bfloat16)
        in_maps.append(m)
    return in_maps


def kernel(**inputs):
    nc = _get_built()
    in_maps = make_in_maps(inputs)
    res = bass_utils.run_bass_kernel_spmd(nc, in_maps, core_ids=list(range(N_CORES)))
    out = np.stack([np.asarray(res.results[n]["out"], np.float32).T.reshape(H, WD, C)
                    for n in range(N_CORES)])
    return out
